# revision 12
# baseline (speedup 1.0000x reference)
"""MoE top-2 routing kernel for Trainium2, 8-core data-parallel.

Problem: x [524288, 128] f32; gate Linear(128->8); 8 experts Linear(128->128).
  g = softmax(x @ gate_W.T + gate_b); top-2 mask; out = sum_e (g*mask)_e * (x @ W_e.T) + g @ b

Strategy (per core, 65536 tokens, 512 tiles of 128 tokens):
  - PE transpose x tile -> xT (f32r), ACT copies PSUM->SBUF
  - gate logits via matmul(lhsT=xT, rhs=gate_W.T) token-major [t, 8]
  - softmax + top-2 on DVE/ACT (max8 gives 2nd max directly)
  - experts: matmul(lhsT=xT f32r, rhs=Wcat f32r [d, 1024]) -> yall PSUM [t, 8*128]
  - weighted reduce: DVE chained scalar_tensor_tensor for blocks 0..4 + ACT
    scaled copies blocks 5..7 merged on DVE; bias term via small PE matmul
    (lhsT=g.T, rhs=b) bf16, added in the final DVE op.
"""

import sys

if "/opt/trn_rl_repo" not in sys.path:
    sys.path.insert(0, "/opt/trn_rl_repo")

from contextlib import ExitStack

import ml_dtypes
import numpy as np

import concourse.bass as bass
import concourse.tile as tile
from concourse import bacc
from concourse import mybir
from concourse.bass import ds
from concourse.masks import make_identity

F32 = mybir.dt.float32
F32R = mybir.dt.float32r
BF16 = mybir.dt.bfloat16
AF = mybir.ActivationFunctionType
OP = mybir.AluOpType

N_TOKENS = 524288
D = 128
E = 8
N_CORES = 8
P = 128  # partitions / tokens per tile

# DVE chain handles experts [0, N_DVE); ACT scaled-copies handle [N_DVE, 8)
N_DVE = 5


def build_nc(shard_tokens: int, inner_tiles: int = 8) -> bass.Bass:
    ntiles = shard_tokens // P
    assert ntiles % inner_tiles == 0
    outer = ntiles // inner_tiles
    tok_per_iter = inner_tiles * P

    nc = bacc.Bacc()
    x = nc.dram_tensor("x", [shard_tokens, D], F32R, kind="ExternalInput")
    # wcat[d, e*128+f] = W[e, f, d]; wcat[d, 1024+e] = gate_W[e, d]
    wcat = nc.dram_tensor("wcat", [D, E * D + E], F32R, kind="ExternalInput")
    gb_rep = nc.dram_tensor("gb_rep", [P, E], F32, kind="ExternalInput")
    b_bf = nc.dram_tensor("b_bf", [E, D], BF16, kind="ExternalInput")
    ident_f = nc.dram_tensor("ident_f", [P, P], F32R, kind="ExternalInput")
    ident_bf = nc.dram_tensor("ident_bf", [P, P], BF16, kind="ExternalInput")
    out = nc.dram_tensor("out", [shard_tokens, D], F32, kind="ExternalOutput")

    x_v = x.rearrange("(n a p) d -> n p a d", p=P, a=inner_tiles)
    out_v = out.rearrange("(n a p) d -> n p a d", p=P, a=inner_tiles)

    with ExitStack() as ctx:
        tc = ctx.enter_context(tile.TileContext(nc))
        consts = ctx.enter_context(tc.tile_pool(name="consts", bufs=1))
        io_pool = ctx.enter_context(tc.tile_pool(name="io", bufs=3))
        work = ctx.enter_context(tc.tile_pool(name="work", bufs=3))
        gates = ctx.enter_context(tc.tile_pool(name="gates", bufs=3))
        psum_y = ctx.enter_context(tc.tile_pool(name="psum_y", bufs=2, space="PSUM"))
        psum_m = ctx.enter_context(tc.tile_pool(name="psum_m", bufs=2, space="PSUM"))

        # ---- constants ----
        wcat_sb = consts.tile([D, E * D + E], F32R)
        nc.gpsimd.dma_start(out=wcat_sb, in_=wcat[:, :])
        gb_sb = consts.tile([P, E], F32)
        nc.gpsimd.dma_start(out=gb_sb, in_=gb_rep[:, :])
        b_sb = consts.tile([E, D], BF16)
        nc.gpsimd.dma_start(out=b_sb, in_=b_bf[:, :])
        ident_r = consts.tile([P, P], F32R)
        nc.gpsimd.dma_start(out=ident_r, in_=ident_f[:, :])
        ident_b = consts.tile([P, P], BF16)
        nc.gpsimd.dma_start(out=ident_b, in_=ident_bf[:, :])

        wmov_r = wcat_sb[:, 0 : E * D]
        wgate_r = wcat_sb[:, E * D : E * D + E]

        def body(base):
            x_in = io_pool.tile([P, inner_tiles, D], F32R, tag="x_in")
            nc.gpsimd.dma_start(out=x_in, in_=x_v[base])
            out_sb = io_pool.tile([P, inner_tiles, D], F32, tag="out_sb")
            for j in range(inner_tiles):
                # misc psum bank: xT 0:128 | logits 128:136 | bias 256:384 | gT [0:8, 384:512]
                mp = psum_m.tile([P, 512], F32, tag="misc")
                yp = psum_y.tile([P, E * D], F32, tag="yall")

                # transpose x tile
                nc.tensor.transpose(mp[:, 0:D].bitcast(F32R), x_in[:, j, :], ident_r)
                xt = work.tile([P, D], F32R, tag="xt")
                nc.scalar.copy(xt, mp[:, 0:D])
                xt_r = xt

                # gate logits
                nc.tensor.matmul(
                    mp[:, 128 : 128 + E], xt_r, wgate_r, start=True, stop=True
                )
                lg = gates.tile([P, E], F32, tag="lg")
                nc.vector.tensor_tensor(out=lg, in0=mp[:, 128 : 128 + E], in1=gb_sb, op=OP.add)

                # softmax pieces
                eg = gates.tile([P, E], F32, tag="eg")
                s = gates.tile([P, 1], F32, tag="s")
                nc.scalar.activation(eg, lg, AF.Exp, accum_out=s)
                t8 = gates.tile([P, 8], F32, tag="t8")
                nc.vector.max(t8, lg)
                r = gates.tile([P, 1], F32, tag="r")
                nc.vector.reciprocal(r, s)
                mk = gates.tile([P, E], F32, tag="mk")
                nc.vector.tensor_scalar(mk, lg, t8[:, 1:2], None, OP.is_ge)
                gu = gates.tile([P, E], F32, tag="gu")
                nc.vector.tensor_scalar(gu, eg, r, None, OP.mult)
                gh = gates.tile([P, E], F32, tag="gh")
                nc.vector.tensor_tensor(out=gh, in0=gu, in1=mk, op=OP.mult)

                # bias term: gT then matmul with b (bf16)
                gu_bf = gates.tile([P, E], BF16, tag="gu_bf")
                nc.vector.tensor_copy(out=gu_bf, in_=gu)
                gt_ps = mp[0:E, 384:512].bitcast(BF16)[:, 0:D]
                nc.tensor.transpose(gt_ps, gu_bf, ident_b)
                gt = gates.tile([E, D], BF16, tag="gt")
                nc.vector.tensor_copy(out=gt, in_=gt_ps)
                nc.tensor.matmul(mp[:, 256:384], gt, b_sb, start=True, stop=True)

                # experts
                nc.tensor.matmul(
                    yp[:, 0:512], xt_r, wmov_r[:, 0:512], start=True, stop=True
                )
                nc.tensor.matmul(
                    yp[:, 512:1024], xt_r, wmov_r[:, 512:1024], start=True, stop=True
                )

                # weighted reduce: DVE chain 0..N_DVE-1
                acc = work.tile([P, D], F32, tag="acc")
                nc.vector.tensor_scalar(acc, yp[:, 0:D], gh[:, 0:1], None, OP.mult)
                for e in range(1, N_DVE):
                    nc.vector.scalar_tensor_tensor(
                        out=acc,
                        in0=yp[:, e * D : (e + 1) * D],
                        scalar=gh[:, e : e + 1],
                        in1=acc,
                        op0=OP.mult,
                        op1=OP.add,
                    )
                # ACT scaled copies for the rest
                cs = work.tile([P, (E - N_DVE) * D], F32, tag="cs")
                for idx, e in enumerate(range(N_DVE, E)):
                    nc.scalar.activation(
                        cs[:, idx * D : (idx + 1) * D],
                        yp[:, e * D : (e + 1) * D],
                        AF.Copy,
                        scale=gh[:, e : e + 1],
                    )
                # merge tree on DVE
                c01 = work.tile([P, D], F32, tag="c01")
                nc.vector.tensor_tensor(
                    out=c01, in0=cs[:, 0:D], in1=cs[:, D : 2 * D], op=OP.add
                )
                nc.vector.tensor_tensor(
                    out=acc, in0=acc, in1=cs[:, 2 * D : 3 * D], op=OP.add
                )
                nc.vector.tensor_tensor(out=acc, in0=acc, in1=c01, op=OP.add)
                # bias add (from psum) -> out staging
                nc.vector.scalar_tensor_tensor(
                    out=out_sb[:, j, :],
                    in0=mp[:, 256:384],
                    scalar=1.0,
                    in1=acc,
                    op0=OP.mult,
                    op1=OP.add,
                )

            nc.gpsimd.dma_start(out=out_v[base], in_=out_sb)

        if outer == 1:
            body(0)
        else:
            with tc.For_i(0, outer, 1) as it:
                body(it)

    nc.compile()
    return nc


def _prep_consts(gate_W, gate_b, W, b):
    wcat = np.concatenate(
        [W.transpose(2, 0, 1).reshape(D, E * D), gate_W.T], axis=1
    ).astype(np.float32)
    ident_f = np.eye(P, dtype=np.float32)
    ident_bf = np.eye(P, dtype=ml_dtypes.bfloat16)
    gb_rep = np.tile(gate_b[None, :].astype(np.float32), (P, 1))
    b_bf = b.astype(ml_dtypes.bfloat16)
    return wcat, gb_rep, b_bf, ident_f, ident_bf


_NC_CACHE = {}


def _get_nc(shard_tokens):
    if shard_tokens not in _NC_CACHE:
        _NC_CACHE[shard_tokens] = build_nc(shard_tokens)
    return _NC_CACHE[shard_tokens]


def kernel(**inputs) -> np.ndarray:
    x = np.ascontiguousarray(np.asarray(inputs["x"], dtype=np.float32))
    gate_W = np.asarray(inputs["gate_W"], dtype=np.float32)
    gate_b = np.asarray(inputs["gate_b"], dtype=np.float32)
    W = np.asarray(inputs["W"], dtype=np.float32)
    b = np.asarray(inputs["b"], dtype=np.float32)

    n = x.shape[0]
    shard = n // N_CORES
    wcat, gb_rep, b_bf, ident_f, ident_bf = _prep_consts(gate_W, gate_b, W, b)

    nc = _get_nc(shard)
    in_maps = [
        {
            "x": x[c * shard : (c + 1) * shard],
            "wcat": wcat,
            "gb_rep": gb_rep,
            "b_bf": b_bf,
            "ident_f": ident_f,
            "ident_bf": ident_bf,
        }
        for c in range(N_CORES)
    ]
    from concourse.bass_utils import run_bass_kernel_spmd

    res = run_bass_kernel_spmd(nc, in_maps, core_ids=list(range(N_CORES)))
    out = np.concatenate([res.results[c]["out"] for c in range(N_CORES)], axis=0)
    return out.astype(np.float32)
